# revision 19
# baseline (speedup 1.0000x reference)
import sys

for _p in ("/opt/trn_rl_repo", "/opt/trn_rl_repo/concourse"):
    if _p not in sys.path:
        sys.path.insert(0, _p)

import numpy as np
import ml_dtypes

N_CORES = 8
B, H, W_DIM, C = 8, 32, 32, 288
N = H * W_DIM          # 1024 points per core (batch-dim sharding: 1 image per core)
O = 64                 # codewords
CHUNK = 512            # PSUM bank free size (fp32)
NCH = N // CHUNK       # 2 chunks

# Per-engine cost (us) of one [<=128, 1024] abs-production tile-op, from HW
# trace: Act 1-op Abs(x*1 - w) = 1.08us; DVE TS-sub (2x mode, 0.46us) + STT
# abs->fp8 (1x, 1.21us) = 1.66us. GPSIMD is excluded: its software
# TensorScalar runs at ~18us per [128,1024] op AND slows co-running DVE ops
# to the same rate (measured lockstep poisoning).
COST_ACT = 1.14
COST_DVE = 1.72

_CACHE = {}


def _patch_drain_split():
    # The end-of-TileContext drain waits on the FULL global clock (engines +
    # one sem per DMA HW queue), overflowing the CTRL_NO struct's sync-wait
    # slots in walrus. Split: emit one 1-wait SP nop per clock component
    # first; the original drain's full-clock add_sem_waits then elides
    # everything via SP wait history.
    import concourse.tile as tile_mod
    from concourse.vector_clock import ScopedClock, VectorClock

    if getattr(tile_mod.TileContext, "_drain_split_patched", False):
        return

    def _drain_and_barrier(self, tick_clock, wait_clock):
        gc = tick_clock.global_clock
        for idx in range(len(gc)):
            tick = gc[idx]
            if tick <= 0:
                continue
            nop = self.nc.sync.nop(nofuse=True, hint="drain_split")
            vc = VectorClock()
            vc.require_at_least(idx, tick)
            wait_clock.add_sem_waits(nop.ins, ScopedClock({None: vc}))
        # Waitless drain: the nops above (same SP sequencer, in order)
        # already guarantee every sem is at its final value here.
        self.nc.sync.drain()
        self.nc.all_engine_barrier()
        assert self.sems is not None
        popped = self.nc._tile_sem_poison_stack.pop()
        assert popped is self._sem_poison
        self.nc.clear_and_free_semaphores(list(self.sems.allocated().values()))
        self.nc.all_engine_barrier()

    tile_mod.TileContext._drain_and_barrier = _drain_and_barrier
    tile_mod.TileContext._drain_split_patched = True


def _assign_units():
    """Static engine assignment for the 72 production units (64 full-o units
    + 8 tail-pair units), each 2 tile-ops, greedily balancing projected
    per-engine finish time."""
    units = [("tail", gp) for gp in range(8)] + [("full", o) for o in range(O)]
    # head-starts: DVE does Zdr memsets + absorbers + end extraction; Act
    # does absorbers + extraction t1s.
    t = {"act": 1.4, "dve": 3.6}
    cost = {"act": 2 * COST_ACT, "dve": 2 * COST_DVE}
    out = []
    for u in units:
        e = min(t, key=lambda k: t[k] + cost[k])
        t[e] += cost[e]
        out.append((u, e))
    return out


def _register_abs_diff():
    # One-uop custom DVE op: out = |in0 - s0| with a per-partition scalar.
    # Halves DVE's per-tile cost vs the native TS-sub + STT-max pair (the
    # native TensorScalar rejects op1=abs_max at ISA encode).
    import numpy as np
    import concourse.dve_ops as dve_ops
    from concourse.dve_spec import Spec, Src0, C0, maxx, lower
    from concourse.dve_spec import _has_src1 as has_src1
    from concourse.dve_uop import DveOpSpec
    from concourse.bass_utils import dve_ver_for

    if hasattr(dve_ops, "ABS_DIFF_ANT"):
        return dve_ops.ABS_DIFF_ANT
    NAME = "ABS_DIFF_ANT"
    SPEC = Spec(
        body=maxx(Src0 - C0, C0 - Src0),
        reference=lambda in0, in1, s0, s1, imm2: np.abs(
            in0.astype(np.float32) - s0
        ),
    )
    ver = dve_ver_for("TRN2")
    row = dve_ops._CUSTOM_DVE_ROW_BASE + len(dve_ops.OPS)
    probe = DveOpSpec(name=NAME, opcode=row, uops=lower(SPEC, ver=ver),
                      rd1_en=has_src1(SPEC))
    op = dve_ops.DveOp(NAME, SPEC, subdim=False, uops_sha={ver: probe.sha(ver)})
    # append in place: bass_utils holds a by-reference import of OPS
    dve_ops.OPS.append(op)
    dve_ops._SUB_OPCODE_FOR_NAME[NAME] = row
    dve_ops.CUSTOM_DVE_SPECS[NAME] = SPEC
    dve_ops.ABS_DIFF_ANT = op
    return op


def _build_program():
    import concourse.bass as bass
    import concourse.tile as tile
    from concourse import mybir

    _patch_drain_split()
    nc = bass.Bass("TRN2", debug=False, num_devices=N_CORES)

    f32 = mybir.dt.float32
    bf16 = mybir.dt.bfloat16
    fp8 = mybir.dt.float8e4
    Abs = mybir.ActivationFunctionType.Abs
    Ident = mybir.ActivationFunctionType.Identity
    DR = mybir.MatmulPerfMode.DoubleRow
    AOP = mybir.AluOpType

    # x transposed per core: rows = channel, cols = point. xa/xb are channel
    # blocks 0:128 / 128:256; xt is channels 256:288 replicated to all four
    # SBUF quadrants so one op covers the channel tail of four codewords.
    # Inputs are packed into 4 dram tensors (4 serial SP DMA enqueues at
    # ~0.6us each shave startup): xa alone (the first production ops need
    # only it), xb|xt merged, all w-derived consts merged, tail routing.
    xa_d = nc.dram_tensor("xa", [128, N], bf16, kind="ExternalInput")
    xbt_d = nc.dram_tensor("xbt", [128, 2 * N], bf16, kind="ExternalInput")
    # wcst cols: 0:144 = -w (Act Abs bias; 128:144 = quadrant-packed tail
    # -w[256+j, 4g+q] at [32q+j, 128+g]), 144:288 = +w (DVE TS subtrahend),
    # 288 = bias b (rows 0:64).
    wcst_d = nc.dram_tensor("wcst", [128, 2 * (2 * O + 16) + 1], f32,
                            kind="ExternalInput")
    # tail routing one-hots: [32q+j, i, 64*gp + 4*(2gp+i)+q] = +1
    ztail_d = nc.dram_tensor("ztail", [128, 2, 8 * O], fp8, kind="ExternalInput")
    out_d = nc.dram_tensor("out_t", [O, N], f32, kind="ExternalOutput")

    xa, xbt = xa_d.ap(), xbt_d.ap()
    wcst, ztail_a, out_t = wcst_d.ap(), ztail_d.ap(), out_d.ap()

    from contextlib import ExitStack

    with tile.TileContext(nc) as tc, ExitStack() as ctx:
        const_pool = ctx.enter_context(tc.tile_pool(name="const", bufs=1))
        # One fresh buffer per production unit (72 x 256KB = 18.4MB SBUF):
        # reusing buffers would add WAW/WAR sem waits on the producing ops,
        # overflowing walrus's single sync-wait slot per instruction.
        prod_pool = ctx.enter_context(tc.tile_pool(name="prod", bufs=72))
        tmp_pool = ctx.enter_context(tc.tile_pool(name="tmp", bufs=1))
        psum_pool = ctx.enter_context(tc.tile_pool(name="ps", bufs=1, space="PSUM"))

        # --- SBUF constants (xa/wcst first: first production needs them) ----
        xa_sb = const_pool.tile([128, N], bf16, name="xa_sb")
        nc.sync.dma_start(xa_sb[:], xa[:, :])
        wcst_sb = const_pool.tile([128, 2 * (2 * O + 16) + 1], f32,
                                  name="wcst_sb")
        nc.sync.dma_start(wcst_sb[:], wcst[:, :])
        xbt_sb = const_pool.tile([128, 2 * N], bf16, name="xbt_sb")
        nc.sync.dma_start(xbt_sb[:], xbt[:, :])
        ztail_sb = const_pool.tile([128, 2, 8 * O], fp8, name="ztail_sb")
        nc.sync.dma_start(ztail_sb[:], ztail_a[:, :, :])
        xb_sb = xbt_sb[:, 0:N]
        xt_sb = xbt_sb[:, N : 2 * N]
        negw_sb = wcst_sb[:, 0 : 2 * O + 16]
        wbf_sb = wcst_sb[:, 2 * O + 16 : 2 * (2 * O + 16)]
        b_sb = wcst_sb[:, 2 * (2 * O + 16) : 2 * (2 * O + 16) + 1]

        # Full-pair routing: all-ones column at absolute col 63 (both k-subs);
        # lhsT slice [:, :, 63-o : 127-o] puts the hot column at local index o.
        zdr = const_pool.tile([128, 2, 128], fp8, name="zdr")
        nc.vector.memset(zdr[:], 0.0)
        nc.vector.memset(zdr[:, :, 63:64], 1.0)

        # --- walrus 1-sync-wait discipline: per-engine absorber ops ---------
        # Each engine's first real op would otherwise need a fresh sem wait
        # per DMA queue it reads from. Absorb each input's DMA sem into the
        # engine's wait history with a cheap scratch op first.
        scr_d = const_pool.tile([1, 16], f32, name="scr_d")
        scr_a = const_pool.tile([1, 16], f32, name="scr_a")

        # DVE production TS ops carry a recurring self-WAR wait (tmp_d), so
        # they cannot also absorb a fresh DMA component: pre-absorb every
        # tensor the DVE stream reads. Act ops are wait-free per-op and only
        # need wcst (the first op's second component) pre-absorbed.
        for k, s in enumerate((wcst_sb, xa_sb, xbt_sb)):
            nc.vector.tensor_scalar_add(scr_d[0:1, k : k + 1], s[0:1, 0:1], 0.0)
        nc.scalar.activation(scr_a[0:1, 0:1], wcst_sb[0:1, 0:1], Abs,
                             bias=wcst_sb[0:1, 0:1])

        tmp_d = tmp_pool.tile([128, N], bf16, name="tmp_d", tag="tmp_d")

        # --- PSUM banks -----------------------------------------------------
        bank = [psum_pool.tile([128, CHUNK], f32, name=f"bank{ch}") for ch in range(NCH)]
        tbank = [psum_pool.tile([128, CHUNK], f32, name=f"tbank{ch}") for ch in range(NCH)]
        tinyb = psum_pool.tile([128, CHUNK], f32, name="tinyb")

        # PE absorbers: load the DVE (zdr memset) and ztail-DMA sems into PE
        # wait history via singleton matmuls before the real DR stream.
        nc.tensor.matmul(tinyb[0:1, 0:1], lhsT=zdr[:, 0, 0:1], rhs=zdr[:, 0, 0:1],
                         start=True, stop=True)
        nc.tensor.matmul(tinyb[0:1, 0:1], lhsT=ztail_sb[:, 0, 0:1],
                         rhs=ztail_sb[:, 0, 0:1], start=True, stop=True)


        def produce(eng, dst, src, col, wpos, wneg):
            if eng == "act":
                nc.scalar.activation(dst, src, Abs,
                                     bias=wneg[:, col : col + 1])
            else:
                nc.vector.tensor_scalar_sub(tmp_d[:], src,
                                            wpos[:, col : col + 1])
                nc.vector.scalar_tensor_tensor(dst, tmp_d[:], -1.0, tmp_d[:],
                                               op0=AOP.mult, op1=AOP.max)

        assignment = _assign_units()
        n_full = sum(1 for (k, _), _ in assignment if k == "full")
        full_done = [0]
        tail_done = [0]

        for (kind, a), eng in assignment:
            if kind == "full":
                o = a
                dt = prod_pool.tile([128, 2, N], fp8, name="dt", tag="u")
                for i, src in enumerate((xa_sb, xb_sb)):
                    produce(eng, dt[:, i, :], src, i * O + o, wbf_sb, negw_sb)
                for ch in range(NCH):
                    nc.tensor.matmul(
                        bank[ch][0:O, :],
                        lhsT=zdr[:, :, 63 - o : 127 - o],
                        rhs=dt[:, :, CHUNK * ch : CHUNK * (ch + 1)],
                        start=(full_done[0] == 0),
                        stop=(full_done[0] == n_full - 1),
                        perf_mode=DR,
                    )
                full_done[0] += 1
            else:
                # Tail units run first, so their banks close mid-stream and
                # extraction's PE wait on them is covered by the (later) full
                # banks' stop wait.
                gp = a
                tt = prod_pool.tile([128, 2, N], fp8, name="tt", tag="u")
                for i in range(2):
                    g = 2 * gp + i
                    produce(eng, tt[:, i, :], xt_sb, 2 * O + g, wbf_sb, negw_sb)
                for ch in range(NCH):
                    nc.tensor.matmul(
                        tbank[ch][0:O, :],
                        lhsT=ztail_sb[:, :, O * gp : O * (gp + 1)],
                        rhs=tt[:, :, CHUNK * ch : CHUNK * (ch + 1)],
                        start=(tail_done[0] == 0),
                        stop=(tail_done[0] == 7),
                        perf_mode=DR,
                    )
                tail_done[0] += 1

        # --- extraction on DVE: t1 = bank + b; out = t1 + tbank -------------
        # (an instruction may read at most ONE non-scalar PSUM input, so the
        # two banks are combined in two steps). The full banks stop last, so
        # t1's PE wait covers the earlier tail-bank stops by monotonicity.
        out_sb = const_pool.tile([O, N], f32, name="out_sb")
        t1 = [const_pool.tile([O, CHUNK], f32, name=f"t1_{ch}") for ch in range(NCH)]
        for ch in range(NCH):
            nc.vector.tensor_scalar_add(t1[ch][:], bank[ch][0:O, :], b_sb[0:O, 0:1])
        for ch in range(NCH):
            nc.vector.scalar_tensor_tensor(
                out_sb[0:O, CHUNK * ch : CHUNK * (ch + 1)],
                t1[ch][:], 0.0, tbank[ch][0:O, :],
                op0=AOP.add, op1=AOP.add,
            )

        nc.sync.dma_start(out_t[:, :], out_sb[:])

    return nc


def _prep_inputs(x, w, b):
    xs = x.reshape(B, N, C).astype(np.float32)
    w = np.asarray(w, dtype=np.float32)
    fp8 = ml_dtypes.float8_e4m3
    bf16 = ml_dtypes.bfloat16

    negw = np.zeros((128, 2 * O + 16), dtype=np.float32)
    for i in range(2):
        negw[:, i * O : (i + 1) * O] = -w[128 * i : 128 * (i + 1), :]
    for g in range(16):
        for q in range(4):
            negw[32 * q : 32 * q + 32, 2 * O + g] = -w[256:288, 4 * g + q]
    wcst = np.zeros((128, 2 * (2 * O + 16) + 1), dtype=np.float32)
    wcst[:, 0 : 2 * O + 16] = negw
    wcst[:, 2 * O + 16 : 2 * (2 * O + 16)] = -negw
    wcst[0:O, 2 * (2 * O + 16)] = np.asarray(b, dtype=np.float32)

    ztail = np.zeros((128, 2, 8 * O), dtype=np.float32)
    for gp in range(8):
        for i in range(2):
            for q in range(4):
                o = 4 * (2 * gp + i) + q
                ztail[32 * q : 32 * q + 32, i, O * gp + o] = 1.0
    ztail = ztail.astype(fp8)

    in_maps = []
    for core in range(N_CORES):
        xT = xs[core].T  # [C, N]
        xa = xT[0:128].astype(bf16)
        xbt = np.concatenate(
            [xT[128:256], np.tile(xT[256:288], (4, 1))], axis=1
        ).astype(bf16)
        in_maps.append({"xa": xa, "xbt": xbt, "wcst": wcst, "ztail": ztail})
    return in_maps


def kernel(x, w, b):
    from concourse.bass_utils import run_bass_kernel_spmd

    if "nc" not in _CACHE:
        _CACHE["nc"] = _build_program()
    nc = _CACHE["nc"]

    in_maps = _prep_inputs(x, w, b)
    res = run_bass_kernel_spmd(nc, in_maps, list(range(N_CORES)))
    out = np.stack(
        [np.asarray(res.results[core]["out_t"], dtype=np.float32).T for core in range(N_CORES)]
    )
    return out.astype(np.float32)


# revision 20
# speedup vs baseline: 1.0132x; 1.0132x over previous
import sys

for _p in ("/opt/trn_rl_repo", "/opt/trn_rl_repo/concourse"):
    if _p not in sys.path:
        sys.path.insert(0, _p)

import numpy as np
import ml_dtypes

N_CORES = 8
B, H, W_DIM, C = 8, 32, 32, 288
N = H * W_DIM          # 1024 points per core (batch-dim sharding: 1 image per core)
O = 64                 # codewords
CHUNK = 512            # PSUM bank free size (fp32)
NCH = N // CHUNK       # 2 chunks

# Per-engine cost (us) of one [<=128, 1024] abs-production tile-op, from HW
# trace: Act 1-op Abs(x*1 - w) = 1.08us; DVE TS-sub (2x mode, 0.46us) + STT
# abs->fp8 (1x, 1.21us) = 1.66us. GPSIMD is excluded: its software
# TensorScalar runs at ~18us per [128,1024] op AND slows co-running DVE ops
# to the same rate (measured lockstep poisoning).
COST_ACT = 1.14
COST_DVE = 1.72

_CACHE = {}


def _patch_drain_split():
    # The end-of-TileContext drain waits on the FULL global clock (engines +
    # one sem per DMA HW queue), overflowing the CTRL_NO struct's sync-wait
    # slots in walrus. Split: emit one 1-wait SP nop per clock component
    # first; the original drain's full-clock add_sem_waits then elides
    # everything via SP wait history.
    import concourse.tile as tile_mod
    from concourse.vector_clock import ScopedClock, VectorClock

    if getattr(tile_mod.TileContext, "_drain_split_patched", False):
        return

    def _drain_and_barrier(self, tick_clock, wait_clock):
        gc = tick_clock.global_clock
        for idx in range(len(gc)):
            tick = gc[idx]
            if tick <= 0:
                continue
            nop = self.nc.sync.nop(nofuse=True, hint="drain_split")
            vc = VectorClock()
            vc.require_at_least(idx, tick)
            wait_clock.add_sem_waits(nop.ins, ScopedClock({None: vc}))
        # Waitless drain: the nops above (same SP sequencer, in order)
        # already guarantee every sem is at its final value here.
        self.nc.sync.drain()
        self.nc.all_engine_barrier()
        assert self.sems is not None
        popped = self.nc._tile_sem_poison_stack.pop()
        assert popped is self._sem_poison
        self.nc.clear_and_free_semaphores(list(self.sems.allocated().values()))
        self.nc.all_engine_barrier()

    tile_mod.TileContext._drain_and_barrier = _drain_and_barrier
    tile_mod.TileContext._drain_split_patched = True


def _assign_units():
    """Static engine assignment for the 72 production units (64 full-o units
    + 8 tail-pair units), each 2 tile-ops, greedily balancing projected
    per-engine finish time."""
    units = ([("full", o) for o in range(16)]
             + [("tail", gp) for gp in range(8)]
             + [("full", o) for o in range(16, O)])
    # head-starts: DVE does Zdr memsets + absorbers + end extraction; Act
    # does absorbers + extraction t1s.
    t = {"act": 1.4, "dve": 3.6}
    cost = {"act": 2 * COST_ACT, "dve": 2 * COST_DVE}
    out = []
    for u in units:
        e = min(t, key=lambda k: t[k] + cost[k])
        t[e] += cost[e]
        out.append((u, e))
    return out


def _register_abs_diff():
    # One-uop custom DVE op: out = |in0 - s0| with a per-partition scalar.
    # Halves DVE's per-tile cost vs the native TS-sub + STT-max pair (the
    # native TensorScalar rejects op1=abs_max at ISA encode).
    import numpy as np
    import concourse.dve_ops as dve_ops
    from concourse.dve_spec import Spec, Src0, C0, maxx, lower
    from concourse.dve_spec import _has_src1 as has_src1
    from concourse.dve_uop import DveOpSpec
    from concourse.bass_utils import dve_ver_for

    if hasattr(dve_ops, "ABS_DIFF_ANT"):
        return dve_ops.ABS_DIFF_ANT
    NAME = "ABS_DIFF_ANT"
    SPEC = Spec(
        body=maxx(Src0 - C0, C0 - Src0),
        reference=lambda in0, in1, s0, s1, imm2: np.abs(
            in0.astype(np.float32) - s0
        ),
    )
    ver = dve_ver_for("TRN2")
    row = dve_ops._CUSTOM_DVE_ROW_BASE + len(dve_ops.OPS)
    probe = DveOpSpec(name=NAME, opcode=row, uops=lower(SPEC, ver=ver),
                      rd1_en=has_src1(SPEC))
    op = dve_ops.DveOp(NAME, SPEC, subdim=False, uops_sha={ver: probe.sha(ver)})
    # append in place: bass_utils holds a by-reference import of OPS
    dve_ops.OPS.append(op)
    dve_ops._SUB_OPCODE_FOR_NAME[NAME] = row
    dve_ops.CUSTOM_DVE_SPECS[NAME] = SPEC
    dve_ops.ABS_DIFF_ANT = op
    return op


def _build_program():
    import concourse.bass as bass
    import concourse.tile as tile
    from concourse import mybir

    _patch_drain_split()
    nc = bass.Bass("TRN2", debug=False, num_devices=N_CORES)

    f32 = mybir.dt.float32
    bf16 = mybir.dt.bfloat16
    fp8 = mybir.dt.float8e4
    Abs = mybir.ActivationFunctionType.Abs
    Ident = mybir.ActivationFunctionType.Identity
    DR = mybir.MatmulPerfMode.DoubleRow
    AOP = mybir.AluOpType

    # x transposed per core: rows = channel, cols = point. xa/xb are channel
    # blocks 0:128 / 128:256; xt is channels 256:288 replicated to all four
    # SBUF quadrants so one op covers the channel tail of four codewords.
    # Inputs are packed into 4 dram tensors (4 serial SP DMA enqueues at
    # ~0.6us each shave startup): xa alone (the first production ops need
    # only it), xb|xt merged, all w-derived consts merged, tail routing.
    xa_d = nc.dram_tensor("xa", [128, N], bf16, kind="ExternalInput")
    xbt_d = nc.dram_tensor("xbt", [128, 2 * N], bf16, kind="ExternalInput")
    # wcst cols: 0:144 = -w (Act Abs bias; 128:144 = quadrant-packed tail
    # -w[256+j, 4g+q] at [32q+j, 128+g]), 144:288 = +w (DVE TS subtrahend),
    # 288 = bias b (rows 0:64).
    wcst_d = nc.dram_tensor("wcst", [128, 2 * (2 * O + 16) + 1], f32,
                            kind="ExternalInput")
    # tail routing one-hots: [32q+j, i, 64*gp + 4*(2gp+i)+q] = +1
    ztail_d = nc.dram_tensor("ztail", [128, 2, 8 * O], fp8, kind="ExternalInput")
    out_d = nc.dram_tensor("out_t", [O, N], f32, kind="ExternalOutput")

    xa, xbt = xa_d.ap(), xbt_d.ap()
    wcst, ztail_a, out_t = wcst_d.ap(), ztail_d.ap(), out_d.ap()

    from contextlib import ExitStack

    with tile.TileContext(nc) as tc, ExitStack() as ctx:
        const_pool = ctx.enter_context(tc.tile_pool(name="const", bufs=1))
        # One fresh buffer per production unit (72 x 256KB = 18.4MB SBUF):
        # reusing buffers would add WAW/WAR sem waits on the producing ops,
        # overflowing walrus's single sync-wait slot per instruction.
        prod_pool = ctx.enter_context(tc.tile_pool(name="prod", bufs=72))
        tmp_pool = ctx.enter_context(tc.tile_pool(name="tmp", bufs=1))
        psum_pool = ctx.enter_context(tc.tile_pool(name="ps", bufs=1, space="PSUM"))

        # --- SBUF constants (xa first: first production units read it) -----
        xa_sb = const_pool.tile([128, N], bf16, name="xa_sb")
        nc.sync.dma_start(xa_sb[:], xa[:, :])
        xbt_sb = const_pool.tile([128, 2 * N], bf16, name="xbt_sb")
        nc.sync.dma_start(xbt_sb[:], xbt[:, :])
        wcst_sb = const_pool.tile([128, 2 * (2 * O + 16) + 1], f32,
                                  name="wcst_sb")
        nc.sync.dma_start(wcst_sb[:], wcst[:, :])
        ztail_sb = const_pool.tile([128, 2, 8 * O], fp8, name="ztail_sb")
        nc.sync.dma_start(ztail_sb[:], ztail_a[:, :, :])
        xb_sb = xbt_sb[:, 0:N]
        xt_sb = xbt_sb[:, N : 2 * N]
        negw_sb = wcst_sb[:, 0 : 2 * O + 16]
        wbf_sb = wcst_sb[:, 2 * O + 16 : 2 * (2 * O + 16)]
        b_sb = wcst_sb[:, 2 * (2 * O + 16) : 2 * (2 * O + 16) + 1]

        # Full-pair routing: all-ones column at absolute col 63 (both k-subs);
        # lhsT slice [:, :, 63-o : 127-o] puts the hot column at local index o.
        zdr = const_pool.tile([128, 2, 128], fp8, name="zdr")
        nc.vector.memset(zdr[:], 0.0)
        nc.vector.memset(zdr[:, :, 63:64], 1.0)

        # --- walrus 1-sync-wait discipline: per-engine absorber ops ---------
        # Each engine's first real op would otherwise need a fresh sem wait
        # per DMA queue it reads from. Absorb each input's DMA sem into the
        # engine's wait history with a cheap scratch op first.
        scr_d = const_pool.tile([1, 16], f32, name="scr_d")
        scr_a = const_pool.tile([1, 16], f32, name="scr_a")

        # DVE production TS ops carry a recurring self-WAR wait (tmp_d), so
        # they cannot also absorb a fresh DMA component: pre-absorb every
        # tensor the DVE stream reads. Act ops are wait-free per-op and only
        # need wcst (the first op's second component) pre-absorbed.
        for k, s in enumerate((wcst_sb, xa_sb, xbt_sb)):
            nc.vector.tensor_scalar_add(scr_d[0:1, k : k + 1], s[0:1, 0:1], 0.0)
        nc.scalar.activation(scr_a[0:1, 0:1], wcst_sb[0:1, 0:1], Abs,
                             bias=wcst_sb[0:1, 0:1])

        tmp_d = tmp_pool.tile([128, N], bf16, name="tmp_d", tag="tmp_d")

        # --- PSUM banks -----------------------------------------------------
        bank = [psum_pool.tile([128, CHUNK], f32, name=f"bank{ch}") for ch in range(NCH)]
        tbank = [psum_pool.tile([128, CHUNK], f32, name=f"tbank{ch}") for ch in range(NCH)]
        tinyb = psum_pool.tile([128, CHUNK], f32, name="tinyb")

        # PE absorbers: load the DVE (zdr memset) and ztail-DMA sems into PE
        # wait history via singleton matmuls before the real DR stream.
        nc.tensor.matmul(tinyb[0:1, 0:1], lhsT=zdr[:, 0, 0:1], rhs=zdr[:, 0, 0:1],
                         start=True, stop=True)
        nc.tensor.matmul(tinyb[0:1, 0:1], lhsT=ztail_sb[:, 0, 0:1],
                         rhs=ztail_sb[:, 0, 0:1], start=True, stop=True)


        def produce(eng, dst, src, col, wpos, wneg):
            if eng == "act":
                nc.scalar.activation(dst, src, Abs,
                                     bias=wneg[:, col : col + 1])
            else:
                nc.vector.tensor_scalar_sub(tmp_d[:], src,
                                            wpos[:, col : col + 1])
                nc.vector.scalar_tensor_tensor(dst, tmp_d[:], -1.0, tmp_d[:],
                                               op0=AOP.mult, op1=AOP.max)

        assignment = _assign_units()
        n_full = sum(1 for (k, _), _ in assignment if k == "full")
        full_done = [0]
        tail_done = [0]

        for (kind, a), eng in assignment:
            if kind == "full":
                o = a
                dt = prod_pool.tile([128, 2, N], fp8, name="dt", tag="u")
                for i, src in enumerate((xa_sb, xb_sb)):
                    produce(eng, dt[:, i, :], src, i * O + o, wbf_sb, negw_sb)
                for ch in range(NCH):
                    nc.tensor.matmul(
                        bank[ch][0:O, :],
                        lhsT=zdr[:, :, 63 - o : 127 - o],
                        rhs=dt[:, :, CHUNK * ch : CHUNK * (ch + 1)],
                        start=(full_done[0] == 0),
                        stop=(full_done[0] == n_full - 1),
                        perf_mode=DR,
                    )
                full_done[0] += 1
            else:
                # Tail units run first, so their banks close mid-stream and
                # extraction's PE wait on them is covered by the (later) full
                # banks' stop wait.
                gp = a
                tt = prod_pool.tile([128, 2, N], fp8, name="tt", tag="u")
                for i in range(2):
                    g = 2 * gp + i
                    produce(eng, tt[:, i, :], xt_sb, 2 * O + g, wbf_sb, negw_sb)
                for ch in range(NCH):
                    nc.tensor.matmul(
                        tbank[ch][0:O, :],
                        lhsT=ztail_sb[:, :, O * gp : O * (gp + 1)],
                        rhs=tt[:, :, CHUNK * ch : CHUNK * (ch + 1)],
                        start=(tail_done[0] == 0),
                        stop=(tail_done[0] == 7),
                        perf_mode=DR,
                    )
                tail_done[0] += 1

        # --- extraction: t1 = bank + b (Act), out = t1 + tbank (DVE) --------
        # (an instruction may read at most ONE non-scalar PSUM input, so the
        # two banks are combined in two steps, split across engines). The DVE
        # touch pre-loads the tail banks' PE stop tick so each STT carries
        # only the Act t1 wait.
        nc.vector.tensor_scalar_add(scr_d[0:1, 4:5], tbank[1][0:1, 0:1], 0.0)
        out_sb = const_pool.tile([O, N], f32, name="out_sb")
        t1 = [const_pool.tile([O, CHUNK], f32, name=f"t1_{ch}") for ch in range(NCH)]
        for ch in range(NCH):
            nc.scalar.activation(t1[ch][:], bank[ch][0:O, :], Ident,
                                 bias=b_sb[0:O, 0:1])
        for ch in range(NCH):
            nc.vector.scalar_tensor_tensor(
                out_sb[0:O, CHUNK * ch : CHUNK * (ch + 1)],
                t1[ch][:], 0.0, tbank[ch][0:O, :],
                op0=AOP.add, op1=AOP.add,
            )

        nc.sync.dma_start(out_t[:, :], out_sb[:])

    return nc


def _prep_inputs(x, w, b):
    xs = x.reshape(B, N, C).astype(np.float32)
    w = np.asarray(w, dtype=np.float32)
    fp8 = ml_dtypes.float8_e4m3
    bf16 = ml_dtypes.bfloat16

    negw = np.zeros((128, 2 * O + 16), dtype=np.float32)
    for i in range(2):
        negw[:, i * O : (i + 1) * O] = -w[128 * i : 128 * (i + 1), :]
    for g in range(16):
        for q in range(4):
            negw[32 * q : 32 * q + 32, 2 * O + g] = -w[256:288, 4 * g + q]
    wcst = np.zeros((128, 2 * (2 * O + 16) + 1), dtype=np.float32)
    wcst[:, 0 : 2 * O + 16] = negw
    wcst[:, 2 * O + 16 : 2 * (2 * O + 16)] = -negw
    wcst[0:O, 2 * (2 * O + 16)] = np.asarray(b, dtype=np.float32)

    ztail = np.zeros((128, 2, 8 * O), dtype=np.float32)
    for gp in range(8):
        for i in range(2):
            for q in range(4):
                o = 4 * (2 * gp + i) + q
                ztail[32 * q : 32 * q + 32, i, O * gp + o] = 1.0
    ztail = ztail.astype(fp8)

    in_maps = []
    for core in range(N_CORES):
        xT = xs[core].T  # [C, N]
        xa = xT[0:128].astype(bf16)
        xbt = np.concatenate(
            [xT[128:256], np.tile(xT[256:288], (4, 1))], axis=1
        ).astype(bf16)
        in_maps.append({"xa": xa, "xbt": xbt, "wcst": wcst, "ztail": ztail})
    return in_maps


def kernel(x, w, b):
    from concourse.bass_utils import run_bass_kernel_spmd

    if "nc" not in _CACHE:
        _CACHE["nc"] = _build_program()
    nc = _CACHE["nc"]

    in_maps = _prep_inputs(x, w, b)
    res = run_bass_kernel_spmd(nc, in_maps, list(range(N_CORES)))
    out = np.stack(
        [np.asarray(res.results[core]["out_t"], dtype=np.float32).T for core in range(N_CORES)]
    )
    return out.astype(np.float32)


# revision 21
# speedup vs baseline: 1.0697x; 1.0557x over previous
import sys

for _p in ("/opt/trn_rl_repo", "/opt/trn_rl_repo/concourse"):
    if _p not in sys.path:
        sys.path.insert(0, _p)

import numpy as np
import ml_dtypes

N_CORES = 8
B, H, W_DIM, C = 8, 32, 32, 288
N = H * W_DIM          # 1024 points per core (batch-dim sharding: 1 image per core)
O = 64                 # codewords
CHUNK = 512            # PSUM bank free size (fp32)
NCH = N // CHUNK       # 2 chunks

# Per-engine cost (us) of one [<=128, 1024] abs-production tile-op, from HW
# trace: Act 1-op Abs(x*1 - w) = 1.08us; DVE TS-sub (2x mode, 0.46us) + STT
# abs->fp8 (1x, 1.21us) = 1.66us. GPSIMD is excluded: its software
# TensorScalar runs at ~18us per [128,1024] op AND slows co-running DVE ops
# to the same rate (measured lockstep poisoning).
COST_ACT = 1.042
COST_DVE = 1.75

_CACHE = {}


def _patch_drain_split():
    # The end-of-TileContext drain waits on the FULL global clock (engines +
    # one sem per DMA HW queue), overflowing the CTRL_NO struct's sync-wait
    # slots in walrus. Split: emit one 1-wait SP nop per clock component
    # first; the original drain's full-clock add_sem_waits then elides
    # everything via SP wait history.
    import concourse.tile as tile_mod
    from concourse.vector_clock import ScopedClock, VectorClock

    if getattr(tile_mod.TileContext, "_drain_split_patched", False):
        return

    def _drain_and_barrier(self, tick_clock, wait_clock):
        gc = tick_clock.global_clock
        for idx in range(len(gc)):
            tick = gc[idx]
            if tick <= 0:
                continue
            nop = self.nc.sync.nop(nofuse=True, hint="drain_split")
            vc = VectorClock()
            vc.require_at_least(idx, tick)
            wait_clock.add_sem_waits(nop.ins, ScopedClock({None: vc}))
        # Waitless drain: the nops above (same SP sequencer, in order)
        # already guarantee every sem is at its final value here.
        self.nc.sync.drain()
        self.nc.all_engine_barrier()
        assert self.sems is not None
        popped = self.nc._tile_sem_poison_stack.pop()
        assert popped is self._sem_poison
        self.nc.clear_and_free_semaphores(list(self.sems.allocated().values()))
        self.nc.all_engine_barrier()

    tile_mod.TileContext._drain_and_barrier = _drain_and_barrier
    tile_mod.TileContext._drain_split_patched = True


def _assign_units():
    """Static engine assignment for the 72 production units (64 full-o units
    + 8 tail-pair units), each 2 tile-ops, greedily balancing projected
    per-engine finish time."""
    units = ([("full", o) for o in range(16)]
             + [("tail", gp) for gp in range(8)]
             + [("full", o) for o in range(16, O)])
    # head-starts: DVE does Zdr memsets + absorbers + end extraction; Act
    # does absorbers + extraction t1s.
    t = {"act": 2.6, "dve": 0.6}
    cost = {"act": 2 * COST_ACT, "dve": 2 * COST_DVE}
    out = []
    for u in units:
        e = min(t, key=lambda k: t[k] + cost[k])
        t[e] += cost[e]
        out.append((u, e))
    return out


def _register_abs_diff():
    # One-uop custom DVE op: out = |in0 - s0| with a per-partition scalar.
    # Halves DVE's per-tile cost vs the native TS-sub + STT-max pair (the
    # native TensorScalar rejects op1=abs_max at ISA encode).
    import numpy as np
    import concourse.dve_ops as dve_ops
    from concourse.dve_spec import Spec, Src0, C0, maxx, lower
    from concourse.dve_spec import _has_src1 as has_src1
    from concourse.dve_uop import DveOpSpec
    from concourse.bass_utils import dve_ver_for

    if hasattr(dve_ops, "ABS_DIFF_ANT"):
        return dve_ops.ABS_DIFF_ANT
    NAME = "ABS_DIFF_ANT"
    SPEC = Spec(
        body=maxx(Src0 - C0, C0 - Src0),
        reference=lambda in0, in1, s0, s1, imm2: np.abs(
            in0.astype(np.float32) - s0
        ),
    )
    ver = dve_ver_for("TRN2")
    row = dve_ops._CUSTOM_DVE_ROW_BASE + len(dve_ops.OPS)
    probe = DveOpSpec(name=NAME, opcode=row, uops=lower(SPEC, ver=ver),
                      rd1_en=has_src1(SPEC))
    op = dve_ops.DveOp(NAME, SPEC, subdim=False, uops_sha={ver: probe.sha(ver)})
    # append in place: bass_utils holds a by-reference import of OPS
    dve_ops.OPS.append(op)
    dve_ops._SUB_OPCODE_FOR_NAME[NAME] = row
    dve_ops.CUSTOM_DVE_SPECS[NAME] = SPEC
    dve_ops.ABS_DIFF_ANT = op
    return op


def _build_program():
    import concourse.bass as bass
    import concourse.tile as tile
    from concourse import mybir

    _patch_drain_split()
    nc = bass.Bass("TRN2", debug=False, num_devices=N_CORES)

    f32 = mybir.dt.float32
    bf16 = mybir.dt.bfloat16
    fp8 = mybir.dt.float8e4
    Abs = mybir.ActivationFunctionType.Abs
    Ident = mybir.ActivationFunctionType.Identity
    DR = mybir.MatmulPerfMode.DoubleRow
    AOP = mybir.AluOpType

    # x transposed per core: rows = channel, cols = point. xa/xb are channel
    # blocks 0:128 / 128:256; xt is channels 256:288 replicated to all four
    # SBUF quadrants so one op covers the channel tail of four codewords.
    # Inputs are packed into 4 dram tensors (4 serial SP DMA enqueues at
    # ~0.6us each shave startup): xa alone (the first production ops need
    # only it), xb|xt merged, all w-derived consts merged, tail routing.
    xa_d = nc.dram_tensor("xa", [128, N], bf16, kind="ExternalInput")
    xbt_d = nc.dram_tensor("xbt", [128, 2 * N], bf16, kind="ExternalInput")
    # wcst cols: 0:144 = -w (Act Abs bias; 128:144 = quadrant-packed tail
    # -w[256+j, 4g+q] at [32q+j, 128+g]), 144:288 = +w (DVE TS subtrahend),
    # 288 = bias b (rows 0:64).
    wcst_d = nc.dram_tensor("wcst", [128, 2 * (2 * O + 16) + 1], f32,
                            kind="ExternalInput")
    # tail routing one-hots: [32q+j, i, 64*gp + 4*(2gp+i)+q] = +1
    ztail_d = nc.dram_tensor("ztail", [128, 2, 8 * O], fp8, kind="ExternalInput")
    out_d = nc.dram_tensor("out_t", [O, N], f32, kind="ExternalOutput")

    xa, xbt = xa_d.ap(), xbt_d.ap()
    wcst, ztail_a, out_t = wcst_d.ap(), ztail_d.ap(), out_d.ap()

    from contextlib import ExitStack

    with tile.TileContext(nc) as tc, ExitStack() as ctx:
        const_pool = ctx.enter_context(tc.tile_pool(name="const", bufs=1))
        # One fresh buffer per production unit (72 x 256KB = 18.4MB SBUF):
        # reusing buffers would add WAW/WAR sem waits on the producing ops,
        # overflowing walrus's single sync-wait slot per instruction.
        prod_pool = ctx.enter_context(tc.tile_pool(name="prod", bufs=72))
        tmp_pool = ctx.enter_context(tc.tile_pool(name="tmp", bufs=1))
        psum_pool = ctx.enter_context(tc.tile_pool(name="ps", bufs=1, space="PSUM"))

        # --- SBUF constants (wcst first: both engines' absorbers and the act
        # table load chain off it while the bigger x transfers stream in) ----
        wcst_sb = const_pool.tile([128, 2 * (2 * O + 16) + 1], f32,
                                  name="wcst_sb")
        nc.sync.dma_start(wcst_sb[:], wcst[:, :])
        xa_sb = const_pool.tile([128, N], bf16, name="xa_sb")
        nc.sync.dma_start(xa_sb[:], xa[:, :])
        xbt_sb = const_pool.tile([128, 2 * N], bf16, name="xbt_sb")
        nc.sync.dma_start(xbt_sb[:], xbt[:, :])
        ztail_sb = const_pool.tile([128, 2, 8 * O], fp8, name="ztail_sb")
        nc.sync.dma_start(ztail_sb[:], ztail_a[:, :, :])
        xb_sb = xbt_sb[:, 0:N]
        xt_sb = xbt_sb[:, N : 2 * N]
        negw_sb = wcst_sb[:, 0 : 2 * O + 16]
        wbf_sb = wcst_sb[:, 2 * O + 16 : 2 * (2 * O + 16)]
        b_sb = wcst_sb[:, 2 * (2 * O + 16) : 2 * (2 * O + 16) + 1]

        # Full-pair routing: all-ones column at absolute col 63 (both k-subs);
        # lhsT slice [:, :, 63-o : 127-o] puts the hot column at local index o.
        zdr = const_pool.tile([128, 2, 128], fp8, name="zdr")
        nc.vector.memset(zdr[:], 0.0)
        nc.vector.memset(zdr[:, :, 63:64], 1.0)

        # --- walrus 1-sync-wait discipline: per-engine absorber ops ---------
        # Each engine's first real op would otherwise need a fresh sem wait
        # per DMA queue it reads from. Absorb each input's DMA sem into the
        # engine's wait history with a cheap scratch op first.
        scr_d = const_pool.tile([1, 16], f32, name="scr_d")
        scr_a = const_pool.tile([1, 16], f32, name="scr_a")

        # DVE production TS ops carry a recurring self-WAR wait (tmp_d), so
        # they cannot also absorb a fresh DMA component: pre-absorb every
        # tensor the DVE stream reads. Act ops are wait-free per-op and only
        # need wcst (the first op's second component) pre-absorbed.
        for k, s in enumerate((wcst_sb, xa_sb, xbt_sb)):
            nc.vector.tensor_scalar_add(scr_d[0:1, k : k + 1], s[0:1, 0:1], 0.0)
        nc.scalar.activation(scr_a[0:1, 0:1], wcst_sb[0:1, 0:1], Abs,
                             bias=wcst_sb[0:1, 0:1])

        tmp_d = tmp_pool.tile([128, N], bf16, name="tmp_d", tag="tmp_d")

        # --- PSUM banks -----------------------------------------------------
        bank = [psum_pool.tile([128, CHUNK], f32, name=f"bank{ch}") for ch in range(NCH)]
        tbank = [psum_pool.tile([128, CHUNK], f32, name=f"tbank{ch}") for ch in range(NCH)]
        tinyb = psum_pool.tile([128, CHUNK], f32, name="tinyb")

        # PE absorbers: load the DVE (zdr memset) and ztail-DMA sems into PE
        # wait history via singleton matmuls before the real DR stream.
        nc.tensor.matmul(tinyb[0:1, 0:1], lhsT=zdr[:, 0, 0:1], rhs=zdr[:, 0, 0:1],
                         start=True, stop=True)
        nc.tensor.matmul(tinyb[0:1, 0:1], lhsT=ztail_sb[:, 0, 0:1],
                         rhs=ztail_sb[:, 0, 0:1], start=True, stop=True)


        def produce(eng, dst, src, col, wpos, wneg):
            if eng == "act":
                nc.scalar.activation(dst, src, Abs,
                                     bias=wneg[:, col : col + 1])
            else:
                nc.vector.tensor_scalar_sub(tmp_d[:], src,
                                            wpos[:, col : col + 1])
                nc.vector.scalar_tensor_tensor(dst, tmp_d[:], -1.0, tmp_d[:],
                                               op0=AOP.mult, op1=AOP.max)

        assignment = _assign_units()
        n_full = sum(1 for (k, _), _ in assignment if k == "full")
        full_done = [0]
        tail_done = [0]

        for (kind, a), eng in assignment:
            if kind == "full":
                o = a
                dt = prod_pool.tile([128, 2, N], fp8, name="dt", tag="u")
                for i, src in enumerate((xa_sb, xb_sb)):
                    produce(eng, dt[:, i, :], src, i * O + o, wbf_sb, negw_sb)
                for ch in range(NCH):
                    nc.tensor.matmul(
                        bank[ch][0:O, :],
                        lhsT=zdr[:, :, 63 - o : 127 - o],
                        rhs=dt[:, :, CHUNK * ch : CHUNK * (ch + 1)],
                        start=(full_done[0] == 0),
                        stop=(full_done[0] == n_full - 1),
                        perf_mode=DR,
                    )
                full_done[0] += 1
            else:
                # Tail units run first, so their banks close mid-stream and
                # extraction's PE wait on them is covered by the (later) full
                # banks' stop wait.
                gp = a
                tt = prod_pool.tile([128, 2, N], fp8, name="tt", tag="u")
                for i in range(2):
                    g = 2 * gp + i
                    produce(eng, tt[:, i, :], xt_sb, 2 * O + g, wbf_sb, negw_sb)
                for ch in range(NCH):
                    nc.tensor.matmul(
                        tbank[ch][0:O, :],
                        lhsT=ztail_sb[:, :, O * gp : O * (gp + 1)],
                        rhs=tt[:, :, CHUNK * ch : CHUNK * (ch + 1)],
                        start=(tail_done[0] == 0),
                        stop=(tail_done[0] == 7),
                        perf_mode=DR,
                    )
                tail_done[0] += 1

        # --- extraction: t1 = bank + b (Act), out = t1 + tbank (DVE) --------
        # (an instruction may read at most ONE non-scalar PSUM input, so the
        # two banks are combined in two steps, split across engines). The DVE
        # touch pre-loads the tail banks' PE stop tick so each STT carries
        # only the Act t1 wait.
        nc.vector.tensor_scalar_add(scr_d[0:1, 4:5], tbank[1][0:1, 0:1], 0.0)
        out_sb = const_pool.tile([O, N], f32, name="out_sb")
        t1 = [const_pool.tile([O, CHUNK], f32, name=f"t1_{ch}") for ch in range(NCH)]
        for ch in range(NCH):
            nc.scalar.activation(t1[ch][:], bank[ch][0:O, :], Ident,
                                 bias=b_sb[0:O, 0:1])
        for ch in range(NCH):
            nc.vector.scalar_tensor_tensor(
                out_sb[0:O, CHUNK * ch : CHUNK * (ch + 1)],
                t1[ch][:], 0.0, tbank[ch][0:O, :],
                op0=AOP.add, op1=AOP.add,
            )

        nc.sync.dma_start(out_t[:, :], out_sb[:])

    return nc


def _prep_inputs(x, w, b):
    xs = x.reshape(B, N, C).astype(np.float32)
    w = np.asarray(w, dtype=np.float32)
    fp8 = ml_dtypes.float8_e4m3
    bf16 = ml_dtypes.bfloat16

    negw = np.zeros((128, 2 * O + 16), dtype=np.float32)
    for i in range(2):
        negw[:, i * O : (i + 1) * O] = -w[128 * i : 128 * (i + 1), :]
    for g in range(16):
        for q in range(4):
            negw[32 * q : 32 * q + 32, 2 * O + g] = -w[256:288, 4 * g + q]
    wcst = np.zeros((128, 2 * (2 * O + 16) + 1), dtype=np.float32)
    wcst[:, 0 : 2 * O + 16] = negw
    wcst[:, 2 * O + 16 : 2 * (2 * O + 16)] = -negw
    wcst[0:O, 2 * (2 * O + 16)] = np.asarray(b, dtype=np.float32)

    ztail = np.zeros((128, 2, 8 * O), dtype=np.float32)
    for gp in range(8):
        for i in range(2):
            for q in range(4):
                o = 4 * (2 * gp + i) + q
                ztail[32 * q : 32 * q + 32, i, O * gp + o] = 1.0
    ztail = ztail.astype(fp8)

    in_maps = []
    for core in range(N_CORES):
        xT = xs[core].T  # [C, N]
        xa = xT[0:128].astype(bf16)
        xbt = np.concatenate(
            [xT[128:256], np.tile(xT[256:288], (4, 1))], axis=1
        ).astype(bf16)
        in_maps.append({"xa": xa, "xbt": xbt, "wcst": wcst, "ztail": ztail})
    return in_maps


def kernel(x, w, b):
    from concourse.bass_utils import run_bass_kernel_spmd

    if "nc" not in _CACHE:
        _CACHE["nc"] = _build_program()
    nc = _CACHE["nc"]

    in_maps = _prep_inputs(x, w, b)
    res = run_bass_kernel_spmd(nc, in_maps, list(range(N_CORES)))
    out = np.stack(
        [np.asarray(res.results[core]["out_t"], dtype=np.float32).T for core in range(N_CORES)]
    )
    return out.astype(np.float32)


# revision 24
# speedup vs baseline: 1.0750x; 1.0050x over previous
import sys

for _p in ("/opt/trn_rl_repo", "/opt/trn_rl_repo/concourse"):
    if _p not in sys.path:
        sys.path.insert(0, _p)

import numpy as np
import ml_dtypes

N_CORES = 8
B, H, W_DIM, C = 8, 32, 32, 288
N = H * W_DIM          # 1024 points per core (batch-dim sharding: 1 image per core)
O = 64                 # codewords
CHUNK = 512            # PSUM bank free size (fp32)
NCH = N // CHUNK       # 2 chunks

# Per-engine cost (us) of one [<=128, 1024] abs-production tile-op, from HW
# trace: Act 1-op Abs(x*1 - w) = 1.08us; DVE TS-sub (2x mode, 0.46us) + STT
# abs->fp8 (1x, 1.21us) = 1.66us. GPSIMD is excluded: its software
# TensorScalar runs at ~18us per [128,1024] op AND slows co-running DVE ops
# to the same rate (measured lockstep poisoning).
COST_ACT = 1.042
COST_DVE = 1.78

_CACHE = {}


def _patch_drain_split():
    # The end-of-TileContext drain waits on the FULL global clock (engines +
    # one sem per DMA HW queue), overflowing the CTRL_NO struct's sync-wait
    # slots in walrus. Split: emit one 1-wait SP nop per clock component
    # first; the original drain's full-clock add_sem_waits then elides
    # everything via SP wait history.
    import concourse.tile as tile_mod
    from concourse.vector_clock import ScopedClock, VectorClock

    if getattr(tile_mod.TileContext, "_drain_split_patched", False):
        return

    def _drain_and_barrier(self, tick_clock, wait_clock):
        gc = tick_clock.global_clock
        for idx in range(len(gc)):
            tick = gc[idx]
            if tick <= 0:
                continue
            nop = self.nc.sync.nop(nofuse=True, hint="drain_split")
            vc = VectorClock()
            vc.require_at_least(idx, tick)
            wait_clock.add_sem_waits(nop.ins, ScopedClock({None: vc}))
        # Waitless drain: the nops above (same SP sequencer, in order)
        # already guarantee every sem is at its final value here.
        self.nc.sync.drain()
        self.nc.all_engine_barrier()
        assert self.sems is not None
        popped = self.nc._tile_sem_poison_stack.pop()
        assert popped is self._sem_poison
        self.nc.clear_and_free_semaphores(list(self.sems.allocated().values()))
        self.nc.all_engine_barrier()

    tile_mod.TileContext._drain_and_barrier = _drain_and_barrier
    tile_mod.TileContext._drain_split_patched = True


def _assign_units():
    """Static engine assignment for the 72 production units (64 full-o units
    + 8 tail-pair units), each 2 tile-ops, greedily balancing projected
    per-engine finish time."""
    units = ([("full", o) for o in range(16)]
             + [("tail", gp) for gp in range(8)]
             + [("full", o) for o in range(16, O)])
    # head-starts: DVE does Zdr memsets + absorbers + end extraction; Act
    # does absorbers + extraction t1s.
    t = {"act": 2.6, "dve": 0.6}
    cost = {"act": 2 * COST_ACT, "dve": 2 * COST_DVE}
    out = []
    for u in units:
        e = min(t, key=lambda k: t[k] + cost[k])
        t[e] += cost[e]
        out.append((u, e))
    return out


def _build_program():
    import concourse.bass as bass
    import concourse.tile as tile
    from concourse import mybir

    _patch_drain_split()
    nc = bass.Bass("TRN2", debug=False, num_devices=N_CORES)

    f32 = mybir.dt.float32
    bf16 = mybir.dt.bfloat16
    fp8 = mybir.dt.float8e4
    Abs = mybir.ActivationFunctionType.Abs
    Ident = mybir.ActivationFunctionType.Identity
    DR = mybir.MatmulPerfMode.DoubleRow
    AOP = mybir.AluOpType

    # x transposed per core: rows = channel, cols = point. xa/xb are channel
    # blocks 0:128 / 128:256; xt is channels 256:288 replicated to all four
    # SBUF quadrants so one op covers the channel tail of four codewords.
    # Inputs are packed into 4 dram tensors (4 serial SP DMA enqueues at
    # ~0.6us each shave startup): xa alone (the first production ops need
    # only it), xb|xt merged, all w-derived consts merged, tail routing.
    xa_d = nc.dram_tensor("xa", [128, N], bf16, kind="ExternalInput")
    xb_d = nc.dram_tensor("xb", [128, N], bf16, kind="ExternalInput")
    xt_d = nc.dram_tensor("xt", [128, N], bf16, kind="ExternalInput")
    # wcst cols: 0:144 = -w (Act Abs bias; 128:144 = quadrant-packed tail
    # -w[256+j, 4g+q] at [32q+j, 128+g]), 144:288 = +w (DVE TS subtrahend),
    # 288 = bias b (rows 0:64).
    wcst_d = nc.dram_tensor("wcst", [128, 2 * (2 * O + 16) + 1], f32,
                            kind="ExternalInput")
    # tail routing one-hots: [32q+j, i, 64*gp + 4*(2gp+i)+q] = +1
    ztail_d = nc.dram_tensor("ztail", [128, 2, 8 * O], fp8, kind="ExternalInput")
    out_d = nc.dram_tensor("out_t", [O, N], f32, kind="ExternalOutput")

    xa, xb, xt = xa_d.ap(), xb_d.ap(), xt_d.ap()
    wcst, ztail_a, out_t = wcst_d.ap(), ztail_d.ap(), out_d.ap()

    from contextlib import ExitStack

    with tile.TileContext(nc) as tc, ExitStack() as ctx:
        const_pool = ctx.enter_context(tc.tile_pool(name="const", bufs=1))
        # One fresh buffer per production unit (72 x 256KB = 18.4MB SBUF):
        # reusing buffers would add WAW/WAR sem waits on the producing ops,
        # overflowing walrus's single sync-wait slot per instruction.
        prod_pool = ctx.enter_context(tc.tile_pool(name="prod", bufs=72))
        tmp_pool = ctx.enter_context(tc.tile_pool(name="tmp", bufs=1))
        psum_pool = ctx.enter_context(tc.tile_pool(name="ps", bufs=1, space="PSUM"))

        # --- SBUF constants (wcst first: both engines' absorbers and the act
        # table load chain off it while the bigger x transfers stream in) ----
        wcst_sb = const_pool.tile([128, 2 * (2 * O + 16) + 1], f32,
                                  name="wcst_sb")
        nc.sync.dma_start(wcst_sb[:], wcst[:, :])
        xa_sb = const_pool.tile([128, N], bf16, name="xa_sb")
        nc.sync.dma_start(xa_sb[:], xa[:, :])
        xb_sb = const_pool.tile([128, N], bf16, name="xb_sb")
        nc.sync.dma_start(xb_sb[:], xb[:, :])
        xt_sb = const_pool.tile([128, N], bf16, name="xt_sb")
        nc.sync.dma_start(xt_sb[:], xt[:, :])
        ztail_sb = const_pool.tile([128, 2, 8 * O], fp8, name="ztail_sb")
        nc.sync.dma_start(ztail_sb[:], ztail_a[:, :, :])
        negw_sb = wcst_sb[:, 0 : 2 * O + 16]
        wbf_sb = wcst_sb[:, 2 * O + 16 : 2 * (2 * O + 16)]
        b_sb = wcst_sb[:, 2 * (2 * O + 16) : 2 * (2 * O + 16) + 1]

        # Full-pair routing: all-ones column at absolute col 63 (both k-subs);
        # lhsT slice [:, :, 63-o : 127-o] puts the hot column at local index o.
        zdr = const_pool.tile([128, 2, 128], fp8, name="zdr")
        nc.vector.memset(zdr[:], 0.0)
        nc.vector.memset(zdr[:, :, 63:64], 1.0)

        # --- walrus 1-sync-wait discipline: per-engine absorber ops ---------
        # Each engine's first real op would otherwise need a fresh sem wait
        # per DMA queue it reads from. Absorb each input's DMA sem into the
        # engine's wait history with a cheap scratch op first.
        scr_d = const_pool.tile([1, 16], f32, name="scr_d")
        scr_a = const_pool.tile([1, 16], f32, name="scr_a")

        # DVE production TS ops carry a recurring self-WAR wait (tmp_d), so
        # they cannot also absorb a fresh DMA component: pre-absorb every
        # tensor the DVE stream reads. Act ops are wait-free per-op and only
        # need wcst (the first op's second component) pre-absorbed.
        for k, s in enumerate((wcst_sb, xa_sb, xb_sb, xt_sb)):
            nc.vector.tensor_scalar_add(scr_d[0:1, k : k + 1], s[0:1, 0:1], 0.0)
        nc.scalar.activation(scr_a[0:1, 0:1], wcst_sb[0:1, 0:1], Abs,
                             bias=wcst_sb[0:1, 0:1])

        tmp_d = tmp_pool.tile([128, N], bf16, name="tmp_d", tag="tmp_d")

        # --- PSUM banks -----------------------------------------------------
        # One bank per chunk: tail DRs join the full DRs' accumulation group
        # (a matmul's zero lhsT columns accumulate zeros into the rows it
        # doesn't own, so interleaving is harmless).
        bank = [psum_pool.tile([128, CHUNK], f32, name=f"bank{ch}") for ch in range(NCH)]
        tinyb = psum_pool.tile([128, CHUNK], f32, name="tinyb")

        # PE absorbers: load the DVE (zdr memset) and ztail-DMA sems into PE
        # wait history via singleton matmuls before the real DR stream.
        nc.tensor.matmul(tinyb[0:1, 0:1], lhsT=zdr[:, 0, 0:1], rhs=zdr[:, 0, 0:1],
                         start=True, stop=True)
        nc.tensor.matmul(tinyb[0:1, 0:1], lhsT=ztail_sb[:, 0, 0:1],
                         rhs=ztail_sb[:, 0, 0:1], start=True, stop=True)


        def produce(eng, dst, src, col, wpos, wneg):
            if eng == "act":
                nc.scalar.activation(dst, src, Abs,
                                     bias=wneg[:, col : col + 1])
            else:
                nc.vector.tensor_scalar_sub(tmp_d[:], src,
                                            wpos[:, col : col + 1])
                nc.vector.scalar_tensor_tensor(dst, tmp_d[:], -1.0, tmp_d[:],
                                               op0=AOP.mult, op1=AOP.max)

        assignment = _assign_units()
        n_units = len(assignment)
        done = [0]

        for (kind, a), eng in assignment:
            dt = prod_pool.tile([128, 2, N], fp8, name="dt", tag="u")
            if kind == "full":
                o = a
                for i, src in enumerate((xa_sb, xb_sb)):
                    produce(eng, dt[:, i, :], src, i * O + o, wbf_sb, negw_sb)
                lhsT = zdr[:, :, 63 - o : 127 - o]
            else:
                gp = a
                for i in range(2):
                    g = 2 * gp + i
                    produce(eng, dt[:, i, :], xt_sb, 2 * O + g, wbf_sb, negw_sb)
                lhsT = ztail_sb[:, :, O * gp : O * (gp + 1)]
            for ch in range(NCH):
                nc.tensor.matmul(
                    bank[ch][0:O, :],
                    lhsT=lhsT,
                    rhs=dt[:, :, CHUNK * ch : CHUNK * (ch + 1)],
                    start=(done[0] == 0),
                    stop=(done[0] == n_units - 1),
                    perf_mode=DR,
                )
            done[0] += 1

        # --- extraction: out[:, ch] = bank[ch] + b, one op per chunk --------
        # chunk0 on DVE and chunk1 on Act run in parallel; each carries only
        # its bank's PE stop wait, and each chunk DMAs out independently.
        out_sb = const_pool.tile([O, N], f32, name="out_sb")
        nc.vector.tensor_scalar_add(out_sb[0:O, 0:CHUNK], bank[0][0:O, :],
                                    b_sb[0:O, 0:1])
        nc.sync.dma_start(out_t[:, 0:CHUNK], out_sb[0:O, 0:CHUNK])
        nc.scalar.activation(out_sb[0:O, CHUNK : 2 * CHUNK], bank[1][0:O, :],
                             Ident, bias=b_sb[0:O, 0:1])
        nc.sync.dma_start(out_t[:, CHUNK : 2 * CHUNK],
                          out_sb[0:O, CHUNK : 2 * CHUNK])

    return nc


def _prep_inputs(x, w, b):
    xs = x.reshape(B, N, C).astype(np.float32)
    w = np.asarray(w, dtype=np.float32)
    fp8 = ml_dtypes.float8_e4m3
    bf16 = ml_dtypes.bfloat16

    negw = np.zeros((128, 2 * O + 16), dtype=np.float32)
    for i in range(2):
        negw[:, i * O : (i + 1) * O] = -w[128 * i : 128 * (i + 1), :]
    for g in range(16):
        for q in range(4):
            negw[32 * q : 32 * q + 32, 2 * O + g] = -w[256:288, 4 * g + q]
    wcst = np.zeros((128, 2 * (2 * O + 16) + 1), dtype=np.float32)
    wcst[:, 0 : 2 * O + 16] = negw
    wcst[:, 2 * O + 16 : 2 * (2 * O + 16)] = -negw
    wcst[0:O, 2 * (2 * O + 16)] = np.asarray(b, dtype=np.float32)

    ztail = np.zeros((128, 2, 8 * O), dtype=np.float32)
    for gp in range(8):
        for i in range(2):
            for q in range(4):
                o = 4 * (2 * gp + i) + q
                ztail[32 * q : 32 * q + 32, i, O * gp + o] = 1.0
    ztail = ztail.astype(fp8)

    in_maps = []
    for core in range(N_CORES):
        xT = xs[core].T  # [C, N]
        in_maps.append({
            "xa": xT[0:128].astype(bf16),
            "xb": xT[128:256].astype(bf16),
            "xt": np.tile(xT[256:288], (4, 1)).astype(bf16),
            "wcst": wcst, "ztail": ztail,
        })
    return in_maps


def kernel(x, w, b):
    from concourse.bass_utils import run_bass_kernel_spmd

    if "nc" not in _CACHE:
        _CACHE["nc"] = _build_program()
    nc = _CACHE["nc"]

    in_maps = _prep_inputs(x, w, b)
    res = run_bass_kernel_spmd(nc, in_maps, list(range(N_CORES)))
    out = np.stack(
        [np.asarray(res.results[core]["out_t"], dtype=np.float32).T for core in range(N_CORES)]
    )
    return out.astype(np.float32)



# revision 25
# speedup vs baseline: 1.0864x; 1.0106x over previous
import sys

for _p in ("/opt/trn_rl_repo", "/opt/trn_rl_repo/concourse"):
    if _p not in sys.path:
        sys.path.insert(0, _p)

import numpy as np
import ml_dtypes

N_CORES = 8
B, H, W_DIM, C = 8, 32, 32, 288
N = H * W_DIM          # 1024 points per core (batch-dim sharding: 1 image per core)
O = 64                 # codewords
CHUNK = 512            # PSUM bank free size (fp32)
NCH = N // CHUNK       # 2 chunks

# Per-engine cost (us) of one [<=128, 1024] abs-production tile-op, from HW
# trace: Act 1-op Abs(x*1 - w) = 1.08us; DVE TS-sub (2x mode, 0.46us) + STT
# abs->fp8 (1x, 1.21us) = 1.66us. GPSIMD is excluded: its software
# TensorScalar runs at ~18us per [128,1024] op AND slows co-running DVE ops
# to the same rate (measured lockstep poisoning).
COST_ACT = 1.042
COST_DVE = 1.83

_CACHE = {}


def _patch_drain_split():
    # The end-of-TileContext drain waits on the FULL global clock (engines +
    # one sem per DMA HW queue), overflowing the CTRL_NO struct's sync-wait
    # slots in walrus. Split: emit one 1-wait SP nop per clock component
    # first; the original drain's full-clock add_sem_waits then elides
    # everything via SP wait history.
    import concourse.tile as tile_mod
    from concourse.vector_clock import ScopedClock, VectorClock

    if getattr(tile_mod.TileContext, "_drain_split_patched", False):
        return

    def _drain_and_barrier(self, tick_clock, wait_clock):
        gc = tick_clock.global_clock
        for idx in range(len(gc)):
            tick = gc[idx]
            if tick <= 0:
                continue
            nop = self.nc.sync.nop(nofuse=True, hint="drain_split")
            vc = VectorClock()
            vc.require_at_least(idx, tick)
            wait_clock.add_sem_waits(nop.ins, ScopedClock({None: vc}))
        # Waitless drain: the nops above (same SP sequencer, in order)
        # already guarantee every sem is at its final value here.
        self.nc.sync.drain()
        self.nc.all_engine_barrier()
        assert self.sems is not None
        popped = self.nc._tile_sem_poison_stack.pop()
        assert popped is self._sem_poison
        self.nc.clear_and_free_semaphores(list(self.sems.allocated().values()))
        self.nc.all_engine_barrier()

    tile_mod.TileContext._drain_and_barrier = _drain_and_barrier
    tile_mod.TileContext._drain_split_patched = True


def _assign_units():
    """Static engine assignment for the 72 production units (64 full-o units
    + 8 tail-pair units), each 2 tile-ops, greedily balancing projected
    per-engine finish time."""
    units = ([("full", o) for o in range(16)]
             + [("tail", gp) for gp in range(8)]
             + [("full", o) for o in range(16, O)])
    # head-starts: DVE does Zdr memsets + absorbers + end extraction; Act
    # does absorbers + extraction t1s.
    t = {"act": 2.6, "dve": 0.6}
    cost = {"act": 2 * COST_ACT, "dve": 2 * COST_DVE}
    out = []
    for u in units:
        e = min(t, key=lambda k: t[k] + cost[k])
        t[e] += cost[e]
        out.append((u, e))
    return out


def _build_program():
    import concourse.bass as bass
    import concourse.tile as tile
    from concourse import mybir

    _patch_drain_split()
    nc = bass.Bass("TRN2", debug=False, num_devices=N_CORES)

    f32 = mybir.dt.float32
    bf16 = mybir.dt.bfloat16
    fp8 = mybir.dt.float8e4
    Abs = mybir.ActivationFunctionType.Abs
    Ident = mybir.ActivationFunctionType.Identity
    DR = mybir.MatmulPerfMode.DoubleRow
    AOP = mybir.AluOpType

    # x transposed per core: rows = channel, cols = point. xa/xb are channel
    # blocks 0:128 / 128:256; xt is channels 256:288 replicated to all four
    # SBUF quadrants so one op covers the channel tail of four codewords.
    # 5 input dram tensors (SP DMA enqueues are ~0.6us each and serial, and
    # with the 2 output DMAs all 7 land on distinct HW queues).
    xa_d = nc.dram_tensor("xa", [128, N], bf16, kind="ExternalInput")
    xb_d = nc.dram_tensor("xb", [128, N], bf16, kind="ExternalInput")
    xt_d = nc.dram_tensor("xt", [128, N], bf16, kind="ExternalInput")
    # wcst cols: 0:144 = -w (Act Abs bias; 128:144 = quadrant-packed tail
    # -w[256+j, 4g+q] at [32q+j, 128+g]), 144:288 = +w (DVE TS subtrahend),
    # 288 = bias b (rows 0:64).
    wcst_d = nc.dram_tensor("wcst", [128, 2 * (2 * O + 16) + 1], f32,
                            kind="ExternalInput")
    # tail routing one-hots: [32q+j, i, 64*gp + 4*(2gp+i)+q] = +1
    ztail_d = nc.dram_tensor("ztail", [128, 2, 8 * O], fp8, kind="ExternalInput")
    out_d = nc.dram_tensor("out_t", [O, N], f32, kind="ExternalOutput")

    xa, xb, xt = xa_d.ap(), xb_d.ap(), xt_d.ap()
    wcst, ztail_a, out_t = wcst_d.ap(), ztail_d.ap(), out_d.ap()

    from contextlib import ExitStack

    with tile.TileContext(nc) as tc, ExitStack() as ctx:
        const_pool = ctx.enter_context(tc.tile_pool(name="const", bufs=1))
        # One fresh buffer per production unit (72 x 256KB = 18.4MB SBUF):
        # reusing buffers would add WAW/WAR sem waits on the producing ops,
        # overflowing walrus's single sync-wait slot per instruction.
        prod_pool = ctx.enter_context(tc.tile_pool(name="prod", bufs=72))
        tmp_pool = ctx.enter_context(tc.tile_pool(name="tmp", bufs=1))
        psum_pool = ctx.enter_context(tc.tile_pool(name="ps", bufs=1, space="PSUM"))

        # --- SBUF constants (wcst first: both engines' absorbers and the act
        # table load chain off it while the bigger x transfers stream in) ----
        wcst_sb = const_pool.tile([128, 2 * (2 * O + 16) + 1], f32,
                                  name="wcst_sb")
        nc.sync.dma_start(wcst_sb[:], wcst[:, :])
        xa_sb = const_pool.tile([128, N], bf16, name="xa_sb")
        nc.sync.dma_start(xa_sb[:], xa[:, :])
        xb_sb = const_pool.tile([128, N], bf16, name="xb_sb")
        nc.sync.dma_start(xb_sb[:], xb[:, :])
        xt_sb = const_pool.tile([128, N], bf16, name="xt_sb")
        nc.sync.dma_start(xt_sb[:], xt[:, :])
        ztail_sb = const_pool.tile([128, 2, 8 * O], fp8, name="ztail_sb")
        nc.sync.dma_start(ztail_sb[:], ztail_a[:, :, :])
        negw_sb = wcst_sb[:, 0 : 2 * O + 16]
        wbf_sb = wcst_sb[:, 2 * O + 16 : 2 * (2 * O + 16)]
        b_sb = wcst_sb[:, 2 * (2 * O + 16) : 2 * (2 * O + 16) + 1]

        # Full-pair routing: all-ones column at absolute col 63 (both k-subs);
        # lhsT slice [:, :, 63-o : 127-o] puts the hot column at local index o.
        zdr = const_pool.tile([128, 2, 128], fp8, name="zdr")
        nc.vector.memset(zdr[:], 0.0)
        nc.vector.memset(zdr[:, :, 63:64], 1.0)

        # --- walrus 1-sync-wait discipline: per-engine absorber ops ---------
        # Each engine's first real op would otherwise need a fresh sem wait
        # per DMA queue it reads from. Absorb each input's DMA sem into the
        # engine's wait history with a cheap scratch op first.
        scr_d = const_pool.tile([1, 16], f32, name="scr_d")
        scr_a = const_pool.tile([1, 16], f32, name="scr_a")

        # DVE production TS ops carry a recurring self-WAR wait (tmp_d), so
        # they cannot also absorb a fresh DMA component: pre-absorb every
        # tensor the DVE stream reads. Act ops are wait-free per-op and only
        # need wcst (the first op's second component) pre-absorbed.
        for k, s in enumerate((wcst_sb, xa_sb, xb_sb, xt_sb)):
            nc.vector.tensor_scalar_add(scr_d[0:1, k : k + 1], s[0:1, 0:1], 0.0)
        nc.scalar.activation(scr_a[0:1, 0:1], wcst_sb[0:1, 0:1], Abs,
                             bias=wcst_sb[0:1, 0:1])

        tmp_d = tmp_pool.tile([128, N], bf16, name="tmp_d", tag="tmp_d")

        # --- PSUM banks -----------------------------------------------------
        # One bank per chunk: tail DRs join the full DRs' accumulation group
        # (a matmul's zero lhsT columns accumulate zeros into the rows it
        # doesn't own, so interleaving is harmless).
        bank = [psum_pool.tile([128, CHUNK], f32, name=f"bank{ch}") for ch in range(NCH)]
        tinyb = psum_pool.tile([128, CHUNK], f32, name="tinyb")

        # PE absorbers: load the DVE (zdr memset) and ztail-DMA sems into PE
        # wait history via singleton matmuls before the real DR stream.
        nc.tensor.matmul(tinyb[0:1, 0:1], lhsT=zdr[:, 0, 0:1], rhs=zdr[:, 0, 0:1],
                         start=True, stop=True)
        nc.tensor.matmul(tinyb[0:1, 0:1], lhsT=ztail_sb[:, 0, 0:1],
                         rhs=ztail_sb[:, 0, 0:1], start=True, stop=True)


        def produce(eng, dst, src, col, wpos, wneg):
            if eng == "act":
                nc.scalar.activation(dst, src, Abs,
                                     bias=wneg[:, col : col + 1])
            else:
                nc.vector.tensor_scalar_sub(tmp_d[:], src,
                                            wpos[:, col : col + 1])
                nc.vector.scalar_tensor_tensor(dst, tmp_d[:], -1.0, tmp_d[:],
                                               op0=AOP.mult, op1=AOP.max)

        assignment = _assign_units()
        n_units = len(assignment)
        done = [0]

        for (kind, a), eng in assignment:
            dt = prod_pool.tile([128, 2, N], fp8, name="dt", tag="u")
            if kind == "full":
                o = a
                for i, src in enumerate((xa_sb, xb_sb)):
                    produce(eng, dt[:, i, :], src, i * O + o, wbf_sb, negw_sb)
                lhsT = zdr[:, :, 63 - o : 127 - o]
            else:
                gp = a
                for i in range(2):
                    g = 2 * gp + i
                    produce(eng, dt[:, i, :], xt_sb, 2 * O + g, wbf_sb, negw_sb)
                lhsT = ztail_sb[:, :, O * gp : O * (gp + 1)]
            for ch in range(NCH):
                nc.tensor.matmul(
                    bank[ch][0:O, :],
                    lhsT=lhsT,
                    rhs=dt[:, :, CHUNK * ch : CHUNK * (ch + 1)],
                    start=(done[0] == 0),
                    stop=(done[0] == n_units - 1),
                    perf_mode=DR,
                )
            done[0] += 1

        # --- extraction: out[:, ch] = bank[ch] + b, one op per chunk --------
        # chunk0 on DVE and chunk1 on Act run in parallel; each carries only
        # its bank's PE stop wait, and each chunk DMAs out independently.
        out_sb = const_pool.tile([O, N], f32, name="out_sb")
        nc.vector.tensor_scalar_add(out_sb[0:O, 0:CHUNK], bank[0][0:O, :],
                                    b_sb[0:O, 0:1])
        nc.sync.dma_start(out_t[:, 0:CHUNK], out_sb[0:O, 0:CHUNK])
        nc.scalar.activation(out_sb[0:O, CHUNK : 2 * CHUNK], bank[1][0:O, :],
                             Ident, bias=b_sb[0:O, 0:1])
        nc.sync.dma_start(out_t[:, CHUNK : 2 * CHUNK],
                          out_sb[0:O, CHUNK : 2 * CHUNK])

    return nc


def _prep_inputs(x, w, b):
    xs = x.reshape(B, N, C).astype(np.float32)
    w = np.asarray(w, dtype=np.float32)
    fp8 = ml_dtypes.float8_e4m3
    bf16 = ml_dtypes.bfloat16

    negw = np.zeros((128, 2 * O + 16), dtype=np.float32)
    for i in range(2):
        negw[:, i * O : (i + 1) * O] = -w[128 * i : 128 * (i + 1), :]
    for g in range(16):
        for q in range(4):
            negw[32 * q : 32 * q + 32, 2 * O + g] = -w[256:288, 4 * g + q]
    wcst = np.zeros((128, 2 * (2 * O + 16) + 1), dtype=np.float32)
    wcst[:, 0 : 2 * O + 16] = negw
    wcst[:, 2 * O + 16 : 2 * (2 * O + 16)] = -negw
    wcst[0:O, 2 * (2 * O + 16)] = np.asarray(b, dtype=np.float32)

    ztail = np.zeros((128, 2, 8 * O), dtype=np.float32)
    for gp in range(8):
        for i in range(2):
            for q in range(4):
                o = 4 * (2 * gp + i) + q
                ztail[32 * q : 32 * q + 32, i, O * gp + o] = 1.0
    ztail = ztail.astype(fp8)

    in_maps = []
    for core in range(N_CORES):
        xT = xs[core].T  # [C, N]
        in_maps.append({
            "xa": xT[0:128].astype(bf16),
            "xb": xT[128:256].astype(bf16),
            "xt": np.tile(xT[256:288], (4, 1)).astype(bf16),
            "wcst": wcst, "ztail": ztail,
        })
    return in_maps


def kernel(x, w, b):
    from concourse.bass_utils import run_bass_kernel_spmd

    if "nc" not in _CACHE:
        _CACHE["nc"] = _build_program()
    nc = _CACHE["nc"]

    in_maps = _prep_inputs(x, w, b)
    res = run_bass_kernel_spmd(nc, in_maps, list(range(N_CORES)))
    out = np.stack(
        [np.asarray(res.results[core]["out_t"], dtype=np.float32).T for core in range(N_CORES)]
    )
    return out.astype(np.float32)



# revision 26
# speedup vs baseline: 1.1611x; 1.0688x over previous
import sys

for _p in ("/opt/trn_rl_repo", "/opt/trn_rl_repo/concourse"):
    if _p not in sys.path:
        sys.path.insert(0, _p)

import numpy as np
import ml_dtypes

N_CORES = 8
B, H, W_DIM, C = 8, 32, 32, 288
NP = H * W_DIM         # 1024 points per image
N = 2 * NP             # 2048 points per core (one image PAIR per core)
O = 64                 # codewords total
OL = 32                # codewords per core (o-half sharding)
CHUNK = 512            # PSUM bank free size (fp32)
NCH = N // CHUNK       # 4 chunks
# Core c handles image pair (2p, 2p+1), p = c % 4, and codeword half
# h = c // 4 (o in [32h, 32h+32)). Doubling the free dim to 2048 amortizes
# the fixed per-op overheads (~280ns Act, ~250ns DVE pair) over 2x columns.

# Per-engine cost (us) of one [<=128, 2048] abs-production op, extrapolated
# from HW-measured 1024-col costs: Act = 2048*0.833ns + ~0.3us; DVE =
# TS-sub (2x) + STT abs->fp8 (1x) = 2048*1.562ns + ~0.25us. GPSIMD excluded
# (its software TensorScalar is ~15us/op and poisons co-running DVE).
COST_ACT = 2.01
COST_DVE = 3.57

_CACHE = {}


def _patch_drain_split():
    # The end-of-TileContext drain waits on the FULL global clock (engines +
    # one sem per DMA HW queue), overflowing the CTRL_NO struct's sync-wait
    # slots in walrus. Split: emit one 1-wait SP nop per clock component
    # first; the original drain's full-clock add_sem_waits then elides
    # everything via SP wait history.
    import concourse.tile as tile_mod
    from concourse.vector_clock import ScopedClock, VectorClock

    if getattr(tile_mod.TileContext, "_drain_split_patched", False):
        return

    def _drain_and_barrier(self, tick_clock, wait_clock):
        gc = tick_clock.global_clock
        for idx in range(len(gc)):
            tick = gc[idx]
            if tick <= 0:
                continue
            nop = self.nc.sync.nop(nofuse=True, hint="drain_split")
            vc = VectorClock()
            vc.require_at_least(idx, tick)
            wait_clock.add_sem_waits(nop.ins, ScopedClock({None: vc}))
        # Waitless drain: the nops above (same SP sequencer, in order)
        # already guarantee every sem is at its final value here.
        self.nc.sync.drain()
        self.nc.all_engine_barrier()
        assert self.sems is not None
        popped = self.nc._tile_sem_poison_stack.pop()
        assert popped is self._sem_poison
        self.nc.clear_and_free_semaphores(list(self.sems.allocated().values()))
        self.nc.all_engine_barrier()

    tile_mod.TileContext._drain_and_barrier = _drain_and_barrier
    tile_mod.TileContext._drain_split_patched = True


def _assign_units():
    """Static engine assignment for the 36 production units (32 full-o units
    + 4 tail-pair units), each two [128, 2048] abs ops, greedily balancing
    projected per-engine finish time."""
    units = ([("full", o) for o in range(8)]
             + [("tail", gp) for gp in range(4)]
             + [("full", o) for o in range(8, OL)])
    # head-starts: DVE does zdr memsets + absorbers + 2 end extractions; Act
    # does its absorber + table load + 2 end extractions.
    t = {"act": 2.6, "dve": 0.6}
    cost = {"act": 2 * COST_ACT, "dve": 2 * COST_DVE}
    out = []
    for u in units:
        e = min(t, key=lambda k: t[k] + cost[k])
        t[e] += cost[e]
        out.append((u, e))
    return out


def _build_program():
    import concourse.bass as bass
    import concourse.tile as tile
    from concourse import mybir

    _patch_drain_split()
    nc = bass.Bass("TRN2", debug=False, num_devices=N_CORES)

    f32 = mybir.dt.float32
    bf16 = mybir.dt.bfloat16
    fp8 = mybir.dt.float8e4
    Abs = mybir.ActivationFunctionType.Abs
    Ident = mybir.ActivationFunctionType.Identity
    DR = mybir.MatmulPerfMode.DoubleRow
    AOP = mybir.AluOpType

    # x transposed per core (2 images side by side): rows = channel, cols =
    # point. xa/xb are channel blocks 0:128 / 128:256; xt is channels
    # 256:288 replicated to all four SBUF quadrants so one op covers the
    # channel tail of four codewords.
    xa_d = nc.dram_tensor("xa", [128, N], bf16, kind="ExternalInput")
    xb_d = nc.dram_tensor("xb", [128, N], bf16, kind="ExternalInput")
    xt_d = nc.dram_tensor("xt", [128, N], bf16, kind="ExternalInput")
    # wcst cols (per o-half): 0:64 = -w for c-blocks (i*32+o), 64:72 =
    # quadrant-packed tail -w[256+j, 4g+q] at [32q+j, 64+g], 72:144 = +w
    # (same layout), col 144 = bias b (rows 0:32).
    WC = 2 * OL + 8
    wcst_d = nc.dram_tensor("wcst", [128, 2 * WC + 1], f32, kind="ExternalInput")
    # tail routing one-hots: [32q+j, i, 32*gp + 4*(2gp+i)+q] = +1
    ztail_d = nc.dram_tensor("ztail", [128, 2, 4 * OL], fp8, kind="ExternalInput")
    out_d = nc.dram_tensor("out_t", [OL, N], f32, kind="ExternalOutput")

    xa, xb, xt = xa_d.ap(), xb_d.ap(), xt_d.ap()
    wcst, ztail_a, out_t = wcst_d.ap(), ztail_d.ap(), out_d.ap()

    from contextlib import ExitStack

    with tile.TileContext(nc) as tc, ExitStack() as ctx:
        const_pool = ctx.enter_context(tc.tile_pool(name="const", bufs=1))
        # One fresh buffer per production unit (36 x 512KB = 18MB SBUF):
        # reusing buffers would add WAW/WAR sem waits on the producing ops,
        # overflowing walrus's single sync-wait slot per instruction.
        prod_pool = ctx.enter_context(tc.tile_pool(name="prod", bufs=36))
        tmp_pool = ctx.enter_context(tc.tile_pool(name="tmp", bufs=1))
        psum_pool = ctx.enter_context(tc.tile_pool(name="ps", bufs=1, space="PSUM"))

        # --- SBUF constants (wcst first: absorbers + act table load chain
        # off it while the bigger x transfers stream in) ----------------------
        wcst_sb = const_pool.tile([128, 2 * WC + 1], f32, name="wcst_sb")
        nc.sync.dma_start(wcst_sb[:], wcst[:, :])
        xa_sb = const_pool.tile([128, N], bf16, name="xa_sb")
        nc.sync.dma_start(xa_sb[:], xa[:, :])
        xb_sb = const_pool.tile([128, N], bf16, name="xb_sb")
        nc.sync.dma_start(xb_sb[:], xb[:, :])
        xt_sb = const_pool.tile([128, N], bf16, name="xt_sb")
        nc.sync.dma_start(xt_sb[:], xt[:, :])
        ztail_sb = const_pool.tile([128, 2, 4 * OL], fp8, name="ztail_sb")
        nc.sync.dma_start(ztail_sb[:], ztail_a[:, :, :])
        negw_sb = wcst_sb[:, 0:WC]
        wbf_sb = wcst_sb[:, WC : 2 * WC]
        b_sb = wcst_sb[:, 2 * WC : 2 * WC + 1]

        # Full-pair routing: all-ones column at absolute col 31 (both k-subs);
        # lhsT slice [:, :, 31-o : 63-o] puts the hot column at local index o.
        zdr = const_pool.tile([128, 2, 64], fp8, name="zdr")
        nc.vector.memset(zdr[:], 0.0)
        nc.vector.memset(zdr[:, :, 31:32], 1.0)

        # --- walrus 1-sync-wait discipline: per-engine absorber ops ---------
        scr_d = const_pool.tile([1, 16], f32, name="scr_d")
        scr_a = const_pool.tile([1, 16], f32, name="scr_a")

        # DVE production TS ops carry a recurring self-WAR wait (tmp_d), so
        # they cannot also absorb a fresh DMA component: pre-absorb every
        # tensor the DVE stream reads. Act ops are wait-free per-op and only
        # need wcst (the first op's second component) pre-absorbed.
        for k, s in enumerate((wcst_sb, xa_sb, xb_sb, xt_sb)):
            nc.vector.tensor_scalar_add(scr_d[0:1, k : k + 1], s[0:1, 0:1], 0.0)
        nc.scalar.activation(scr_a[0:1, 0:1], wcst_sb[0:1, 0:1], Abs,
                             bias=wcst_sb[0:1, 0:1])

        tmp_d = tmp_pool.tile([128, N], bf16, name="tmp_d", tag="tmp_d")

        # --- PSUM banks: one accumulation group per 512-col chunk -----------
        bank = [psum_pool.tile([128, CHUNK], f32, name=f"bank{ch}")
                for ch in range(NCH)]
        tinyb = psum_pool.tile([128, CHUNK], f32, name="tinyb")

        # PE absorbers: load the DVE (zdr memset) and ztail-DMA sems into PE
        # wait history via singleton matmuls before the real DR stream.
        nc.tensor.matmul(tinyb[0:1, 0:1], lhsT=zdr[:, 0, 0:1], rhs=zdr[:, 0, 0:1],
                         start=True, stop=True)
        nc.tensor.matmul(tinyb[0:1, 0:1], lhsT=ztail_sb[:, 0, 0:1],
                         rhs=ztail_sb[:, 0, 0:1], start=True, stop=True)

        def produce(eng, dst, src, col):
            if eng == "act":
                nc.scalar.activation(dst, src, Abs,
                                     bias=negw_sb[:, col : col + 1])
            else:
                nc.vector.tensor_scalar_sub(tmp_d[:], src,
                                            wbf_sb[:, col : col + 1])
                nc.vector.scalar_tensor_tensor(dst, tmp_d[:], -1.0, tmp_d[:],
                                               op0=AOP.mult, op1=AOP.max)

        assignment = _assign_units()
        n_units = len(assignment)
        done = [0]

        for (kind, a), eng in assignment:
            dt = prod_pool.tile([128, 2, N], fp8, name="dt", tag="u")
            if kind == "full":
                o = a
                for i, src in enumerate((xa_sb, xb_sb)):
                    produce(eng, dt[:, i, :], src, i * OL + o)
                lhsT = zdr[:, :, 31 - o : 63 - o]
            else:
                gp = a
                for i in range(2):
                    produce(eng, dt[:, i, :], xt_sb, 2 * OL + 2 * gp + i)
                lhsT = ztail_sb[:, :, OL * gp : OL * (gp + 1)]
            for ch in range(NCH):
                nc.tensor.matmul(
                    bank[ch][0:OL, :],
                    lhsT=lhsT,
                    rhs=dt[:, :, CHUNK * ch : CHUNK * (ch + 1)],
                    start=(done[0] == 0),
                    stop=(done[0] == n_units - 1),
                    perf_mode=DR,
                )
            done[0] += 1

        # --- extraction: out[:, ch] = bank[ch] + b, one op per chunk --------
        # chunks 0,1 on DVE and 2,3 on Act run in parallel; each carries only
        # its bank's PE stop wait; each engine's half DMAs out independently.
        out_sb = const_pool.tile([OL, N], f32, name="out_sb")
        for ch in (0, 1):
            nc.vector.tensor_scalar_add(
                out_sb[0:OL, CHUNK * ch : CHUNK * (ch + 1)],
                bank[ch][0:OL, :], b_sb[0:OL, 0:1])
        nc.sync.dma_start(out_t[:, 0 : 2 * CHUNK], out_sb[0:OL, 0 : 2 * CHUNK])
        for ch in (2, 3):
            nc.scalar.activation(
                out_sb[0:OL, CHUNK * ch : CHUNK * (ch + 1)],
                bank[ch][0:OL, :], Ident, bias=b_sb[0:OL, 0:1])
        nc.sync.dma_start(out_t[:, 2 * CHUNK : 4 * CHUNK],
                          out_sb[0:OL, 2 * CHUNK : 4 * CHUNK])

    return nc


def _prep_inputs(x, w, b):
    xs = x.reshape(B, NP, C).astype(np.float32)
    w = np.asarray(w, dtype=np.float32)
    b = np.asarray(b, dtype=np.float32)
    fp8 = ml_dtypes.float8_e4m3
    bf16 = ml_dtypes.bfloat16
    WC = 2 * OL + 8

    wcsts = []
    for h in range(2):
        wh = w[:, OL * h : OL * (h + 1)]  # [288, 32]
        negw = np.zeros((128, WC), dtype=np.float32)
        for i in range(2):
            negw[:, i * OL : (i + 1) * OL] = -wh[128 * i : 128 * (i + 1), :]
        for g in range(8):
            for q in range(4):
                negw[32 * q : 32 * q + 32, 2 * OL + g] = -wh[256:288, 4 * g + q]
        wcst = np.zeros((128, 2 * WC + 1), dtype=np.float32)
        wcst[:, 0:WC] = negw
        wcst[:, WC : 2 * WC] = -negw
        wcst[0:OL, 2 * WC] = b[OL * h : OL * (h + 1)]
        wcsts.append(wcst)

    ztail = np.zeros((128, 2, 4 * OL), dtype=np.float32)
    for gp in range(4):
        for i in range(2):
            for q in range(4):
                o = 4 * (2 * gp + i) + q
                ztail[32 * q : 32 * q + 32, i, OL * gp + o] = 1.0
    ztail = ztail.astype(fp8)

    in_maps = []
    for core in range(N_CORES):
        p, h = core % 4, core // 4
        xT = np.concatenate([xs[2 * p].T, xs[2 * p + 1].T], axis=1)  # [C, 2048]
        in_maps.append({
            "xa": xT[0:128].astype(bf16),
            "xb": xT[128:256].astype(bf16),
            "xt": np.tile(xT[256:288], (4, 1)).astype(bf16),
            "wcst": wcsts[h], "ztail": ztail,
        })
    return in_maps


def kernel(x, w, b):
    from concourse.bass_utils import run_bass_kernel_spmd

    if "nc" not in _CACHE:
        _CACHE["nc"] = _build_program()
    nc = _CACHE["nc"]

    in_maps = _prep_inputs(x, w, b)
    res = run_bass_kernel_spmd(nc, in_maps, list(range(N_CORES)))
    out = np.empty((B, NP, O), dtype=np.float32)
    for core in range(N_CORES):
        p, h = core % 4, core // 4
        r = np.asarray(res.results[core]["out_t"], dtype=np.float32)  # [OL, 2048]
        out[2 * p, :, OL * h : OL * (h + 1)] = r[:, 0:NP].T
        out[2 * p + 1, :, OL * h : OL * (h + 1)] = r[:, NP : 2 * NP].T
    return out
